# revision 1
# baseline (speedup 1.0000x reference)
"""Locally-connected 3x3 block (LCBlock) Trainium2 kernel.

Computes out = ELU(einsum('ocdkij,bcdkij->boij', weights, unfold(x)))
for x:[16,32,64,64] f32, weights:[32,32,3,3,64,64] f32.

Strategy (8 NeuronCores, SPMD, no collectives):
  - Spatially shard H=64 into 8 strips of 8 rows; each core gets its strip's
    per-position weights (they shard perfectly) and a 10-row halo'd slab of x.
  - Per position p=(y,x) the LC contraction is a tiny matmul
    [B=16, CK=288] x [CK=288, O=32].  We run it on the PE as 3 PSUM-accumulated
    matmuls (one per dj kernel column): lhsT = patch [K=96=(3di x 32c), M=16b]
    (cheap LDWEIGHTS: cost scales with columns=16), rhs = weights
    [96, 32o] (the big tensor streams as the moving operand).  4 positions run
    concurrently in the 4 PE column-groups via tile_position.
  - bf16 operands (fp32 PSUM accumulation) halve the HBM roofline.
  - ELU = max(x, exp(min(x,0))-1): 2 DVE ops + 1 ACT op per row-wave.
Host side packs/scatters inputs and gathers the 8 output strips.
"""

import os
import sys

import numpy as np

for _p in ("/opt/trn_rl_repo", "/root/.axon_site/_ro/trn_rl_repo"):
    if os.path.isdir(_p) and _p not in sys.path:
        sys.path.insert(0, _p)

import ml_dtypes

import concourse.bacc as bacc
import concourse.mybir as mybir
import concourse.tile as tile
from concourse.bass_interp import get_hw_module
from concourse.bass_utils import run_bass_kernel_spmd

BF16 = ml_dtypes.bfloat16

# Problem shape (hardcoded per contract).
B, C, O, H, W = 16, 32, 32, 64, 64
NCORES = 8
HL = H // NCORES  # local rows per core
KW = 3  # conv kernel size
PART = KW * C  # 96 partitions: (di, c)
XW = W + 2  # padded row width
XFREE = HL * XW * B  # x slab free elems/partition
WCH = 4 * 16 * KW * O  # weight elems/partition per row-wave (j, pbl, dj, o)
WFREE = HL * WCH
OUTF = HL * 16 * O  # out free elems/partition: (w, pbl, o)

_CACHE = {}


def _build(hw=True, reps=1, variant="full", loop_n=None, rpw=1, wbufs=None,
           x2=False, so2=False, ring2=False, walt=False, wsplit=False):
    nc = bacc.Bacc(
        "TRN2", target_bir_lowering=False, debug=False, num_devices=NCORES
    )
    xs_d = nc.dram_tensor("xs", [PART, XFREE], mybir.dt.bfloat16, kind="ExternalInput")
    w_d = nc.dram_tensor("w", [PART, WFREE], mybir.dt.bfloat16, kind="ExternalInput")
    out_d = nc.dram_tensor("out", [4, 16, OUTF], mybir.dt.float32, kind="ExternalOutput")

    if wbufs is None:
        wbufs = {1: 3, 2: 3, 4: 2, 8: 1}[rpw]
    with tile.TileContext(nc) as tc:
        with (
            tc.tile_pool(name="xp", bufs=1) as xp,
            tc.tile_pool(name="wp", bufs=wbufs) as wp,
            tc.tile_pool(name="pp", bufs=3, space="PSUM") as pp,
            tc.tile_pool(name="op", bufs=1) as op,
            tc.tile_pool(name="tp", bufs=2) as tp,
        ):
          import contextlib

          loop_cm = tc.For_i(0, loop_n, 1) if loop_n else contextlib.nullcontext()
          with loop_cm:
           for _rep in range(reps):
            eng2 = nc.scalar if ring2 else nc.sync
            x_t = xp.tile([PART, XFREE], mybir.dt.bfloat16, tag="x")
            if x2:
                cut = 2 * XW * B
                eng2.dma_start(x_t[:, :cut], xs_d[:][:, :cut])
                eng2.dma_start(x_t[:, cut:], xs_d[:][:, cut:])
            else:
                eng2.dma_start(x_t[:], xs_d[:])
            out_t = op.tile([128, OUTF], mybir.dt.float32, tag="o")

            for wg in range(HL // rpw):  # rpw image rows per DMA chunk
              w_t = wp.tile([PART, rpw * WCH], mybir.dt.bfloat16, tag="w")
              wlo = wg * rpw * WCH
              if wsplit:
                  h = rpw * WCH // 2
                  nc.sync.dma_start(w_t[:, :h], w_d[:][:, wlo:wlo + h])
                  nc.scalar.dma_start(
                      w_t[:, h:], w_d[:][:, wlo + h:wlo + 2 * h]
                  )
              else:
                  weng = (nc.sync, nc.scalar)[wg % 2] if walt else nc.sync
                  weng.dma_start(w_t[:], w_d[:][:, wlo:wlo + rpw * WCH])
              for r in range(rpw):
                wv = wg * rpw + r
                ps = pp.tile([128, 512], mybir.dt.float32, tag="ps")
                # zero-fill: matmuls pure-accumulate (start=False) onto this,
                # and ELU reads rows the col-tiled matmuls never touch
                nc.vector.memset(ps[:], 0.0)
                if variant != "dma_only":
                    # one MM per (xx, j): patch col xx serves dj=0,1,2 for
                    # positions x = xx, xx-1, xx-2 (adjacent PSUM slots)
                    coff = 0
                    for xx in range(18):
                        x_lo, x_hi = max(0, xx - 2), min(15, xx)
                        n = x_hi - x_lo + 1
                        for j in range(4):
                            lo = (wv * XW + 16 * j + xx) * B
                            nc.tensor.matmul(
                                ps[32 * j:32 * j + B,
                                   32 * x_lo:32 * (x_hi + 1)],
                                x_t[:, lo:lo + B],
                                w_t[:, r * WCH + coff + j * n * O:
                                     r * WCH + coff + (j + 1) * n * O],
                                start=False,
                                stop=True,
                                skip_group_check=True,
                                tile_position=(0, 32 * j),
                            )
                        coff += 4 * n * O
                if variant in ("full",):
                    # ELU: out = max(psum, exp(min(psum, 0)) - 1)
                    t1 = tp.tile([128, 512], mybir.dt.float32, tag="t1")
                    nc.vector.tensor_scalar_min(t1[:], ps[:], 0.0)
                    nc.scalar.activation(
                        t1[:], t1[:], mybir.ActivationFunctionType.Exp
                    )
                    nc.vector.scalar_tensor_tensor(
                        out_t[:, wv * 512:(wv + 1) * 512],
                        t1[:],
                        -1.0,
                        ps[:],
                        op0=mybir.AluOpType.add,
                        op1=mybir.AluOpType.max,
                    )
                else:
                    # cheap evacuation so deps/out exist: copy psum -> out
                    nc.vector.tensor_copy(
                        out_t[:, wv * 512:(wv + 1) * 512], ps[:]
                    )
                if so2 and wv == HL // 2 - 1:
                    oap = out_d.ap()
                    half = (HL // 2) * 512
                    for j in range(4):
                        eng2.dma_start(
                            oap[j][:, :half], out_t[32 * j:32 * j + 16, :half]
                        )
            oap = out_d.ap()
            half = (HL // 2) * 512 if so2 else 0
            for j in range(4):
                eng2.dma_start(
                    oap[j][:, half:], out_t[32 * j:32 * j + 16, half:]
                )

    nc.compile()
    if hw:
        nc.m = get_hw_module(nc.m)
    return nc


def _pack_inputs(x, weights):
    """Host-side scatter: per-core bf16 slabs."""
    xpad = np.pad(x, ((0, 0), (0, 0), (1, 1), (1, 1))).astype(BF16)  # [B,C,66,66]
    wb = np.asarray(weights).astype(BF16)  # [O,C,3,3,H,W]
    in_maps = []
    for k in range(NCORES):
        # x slab: [di*32+c, y, xx, b] = xpad[b, c, 8k+y+di, xx]
        slabs = [
            np.transpose(xpad[:, :, 8 * k + di:8 * k + di + HL, :], (1, 2, 3, 0))
            for di in range(KW)
        ]
        xs_k = np.ascontiguousarray(np.stack(slabs, 0)).reshape(PART, XFREE)
        # weights, merged-xx layout: per (y, xx, j), 32-col blocks for
        # x = x_lo..x_hi ascending (dj = xx-x descending):
        #   block = W[o, c, di, dj, 8k+y, 16j+x] as [di*32+c, y, o]
        wc = np.transpose(
            wb[:, :, :, :, 8 * k:8 * (k + 1), :], (2, 1, 3, 4, 5, 0)
        )  # [di, c, dj, y, x, o]
        wc = wc.reshape(PART, KW, HL, W, O)  # [(di,c), dj, y, x, o]
        w_k = np.empty((PART, HL, WCH), dtype=BF16)
        coff = 0
        for xx in range(18):
            x_lo, x_hi = max(0, xx - 2), min(15, xx)
            n = x_hi - x_lo + 1
            for j in range(4):
                for t, xr in enumerate(range(x_lo, x_hi + 1)):
                    dj = xx - xr
                    c0 = coff + j * n * O + t * O
                    # [(di,c), y, o]
                    w_k[:, :, c0:c0 + O] = wc[:, dj, :, 16 * j + xr, :]
            coff += 4 * n * O
        in_maps.append({"xs": xs_k, "w": w_k.reshape(PART, WFREE)})
    return in_maps


def _unpack_outputs(results):
    out = np.empty((B, O, H, W), dtype=np.float32)
    for k in range(NCORES):
        arr = results[k]["out"].reshape(4, 16, HL, 16, O)  # [j, b, w, slot, o]
        strip = np.transpose(arr, (1, 4, 2, 0, 3)).reshape(B, O, HL, W)
        out[:, :, 8 * k:8 * (k + 1), :] = strip
    return out


def run(x, weights, trace=False):
    if "nc" not in _CACHE:
        _CACHE["nc"] = _build()
    nc = _CACHE["nc"]
    in_maps = _pack_inputs(np.asarray(x), np.asarray(weights))
    res = run_bass_kernel_spmd(nc, in_maps, list(range(NCORES)), trace=trace)
    return _unpack_outputs(res.results), res


def kernel(x, weights):
    out, _ = run(x, weights)
    return out



# revision 29
# speedup vs baseline: 263186.8617x; 263186.8617x over previous
"""Locally-connected 3x3 block (LCBlock) Trainium2 kernel.

Computes out = ELU(einsum('ocdkij,bcdkij->boij', weights, unfold(x)))
for x:[16,32,64,64] f32, weights:[32,32,3,3,64,64] f32.

Strategy (8 NeuronCores, SPMD, no collectives):
  - Spatially shard H=64 into 8 strips of 8 rows; each core gets its strip's
    per-position weights (they shard perfectly) and a 10-row halo'd slab of x.
  - Per position p=(y,x) the LC contraction is a tiny matmul
    [B=16, CK=288] x [CK=288, O=32].  We run it on the PE as 3 PSUM-accumulated
    matmuls (one per dj kernel column): lhsT = patch [K=96=(3di x 32c), M=16b]
    (cheap LDWEIGHTS: cost scales with columns=16), rhs = weights
    [96, 32o] (the big tensor streams as the moving operand).  4 positions run
    concurrently in the 4 PE column-groups via tile_position.
  - Weights stream as fp8 e3m4 (x128 host-side scale; patch bf16 carries the
    1/128 descale) -> halves the dominant HBM stream vs bf16.  Output in bf16.
  - All input DMAs issue up front (whole fp8 slab fits SBUF) so the stream
    never waits on compute; stores overlap on the scalar HWDGE queue.
  - ELU = max(x, exp(min(x,0))-1): 2 DVE ops + 1 ACT op per row-wave.
Host side packs/scatters inputs and gathers the 8 output strips.
"""

import os
import sys

import numpy as np

for _p in ("/opt/trn_rl_repo", "/root/.axon_site/_ro/trn_rl_repo"):
    if os.path.isdir(_p) and _p not in sys.path:
        sys.path.insert(0, _p)

import ml_dtypes

import concourse.bacc as bacc
import concourse.mybir as mybir
import concourse.tile as tile
from concourse.bass_interp import get_hw_module
from concourse.bass_utils import run_bass_kernel_spmd

BF16 = ml_dtypes.bfloat16
F8E3 = ml_dtypes.float8_e3m4
WSCALE = 128.0  # weights *128 into e3m4 normal range; patch /128 compensates

# Problem shape (hardcoded per contract).
B, C, O, H, W = 16, 32, 32, 64, 64
NCORES = 8
HL = H // NCORES  # local rows per core
KW = 3  # conv kernel size
PART = KW * C  # 96 partitions: (di, c)
XW = W + 2  # padded row width
XFREE = HL * XW * B  # x slab free elems/partition
WCH = 4 * 16 * KW * O  # weight elems/partition per row-wave (j, pbl, dj, o)
WFREE = HL * WCH
OUTF = HL * 16 * O  # out free elems/partition: (w, pbl, o)

_CACHE = {}


def _build(hw=True, chunks=(1, 1, 1, 1, 1, 1, 1, 1), wdt="f8e3", odt="bf16",
           upfront=True, sowv=5, store_eng="sync", ms_eng="vector", reps=1,
           xxs=1, elu2=True, xsplit=4):
    w_dt = {"f8e3": mybir.dt.float8e3, "bf16": mybir.dt.bfloat16}[wdt]
    o_dt = {"bf16": mybir.dt.bfloat16, "f32": mybir.dt.float32}[odt]
    nc = bacc.Bacc(
        "TRN2", target_bir_lowering=False, debug=False, num_devices=NCORES
    )
    xs_d = nc.dram_tensor("xs", [PART, XFREE], mybir.dt.bfloat16, kind="ExternalInput")
    w_d = nc.dram_tensor("w", [PART, WFREE], w_dt, kind="ExternalInput")
    out_d = nc.dram_tensor("out", [4, 16, OUTF], o_dt, kind="ExternalOutput")

    assert sum(chunks) == HL
    nchunk = len(chunks)
    cstart = [sum(chunks[:g]) for g in range(nchunk + 1)]
    with tile.TileContext(nc) as tc:
        with (
            tc.tile_pool(name="xp", bufs=1) as xp,
            tc.tile_pool(name="wp", bufs=(1 if upfront else 3)) as wp,
            tc.tile_pool(name="pp", bufs=1, space="PSUM") as pp,
            tc.tile_pool(name="op", bufs=1) as op,
            tc.tile_pool(name="tp", bufs=3) as tp,
        ):
          for _rep in range(reps):
            seng = {"scalar": nc.scalar, "sync": nc.sync}[store_eng]
            mseng = {"vector": nc.vector, "gpsimd": nc.gpsimd,
                     "scalar": nc.scalar}[ms_eng]
            # x split into xsplit tiles, each DMA'd just before the first w
            # chunk whose waves need it: waves start as early as possible
            xwaves = HL // xsplit  # waves per x tile
            xh = XFREE // xsplit
            x_ts = [xp.tile([PART, xh], mybir.dt.bfloat16, tag=f"x{h}",
                            name=f"x{h}")
                    for h in range(xsplit)]
            xdone = 0

            def _need_x(upto):
                nonlocal xdone
                while xdone * xwaves < upto:
                    nc.sync.dma_start(
                        x_ts[xdone][:],
                        xs_d[:][:, xdone * xh:(xdone + 1) * xh],
                    )
                    xdone += 1

            out_t = op.tile([128, OUTF], o_dt, tag="o")

            w_ts = []
            for wg in range(nchunk):
                _need_x(cstart[wg] + 1)
                w_t = wp.tile([PART, chunks[wg] * WCH], w_dt,
                              tag=f"w{wg}", name=f"w{wg}")
                wlo = cstart[wg] * WCH
                nc.sync.dma_start(
                    w_t[:], w_d[:][:, wlo:wlo + chunks[wg] * WCH]
                )
                w_ts.append(w_t)
            _need_x(HL)

            # all PSUM banks zeroed up front: per-wave matmuls never wait on
            # a memset queued behind the previous wave's ELU ops
            ps_ts = []
            for wv in range(HL):
                ps = pp.tile([128, 512], mybir.dt.float32, tag=f"ps{wv}",
                             name=f"ps{wv}")
                mseng.memset(ps[:], 0.0)
                ps_ts.append(ps)

            for wg in range(nchunk):
              w_t = w_ts[wg]
              for r in range(chunks[wg]):
                wv = cstart[wg] + r
                ps = ps_ts[wv]
                x_t = x_ts[wv // xwaves]
                xwv = wv % xwaves
                # one MM per (xx, j): patch col xx serves dj=0,1,2 for
                # positions x = xx, xx-1, xx-2 (adjacent PSUM slots).
                # stride-xxs xx order keeps consecutive same-quadrant MMs on
                # disjoint PSUM columns (no overlapping-accumulate drains)
                coffs = [0] * 19
                for xx in range(18):
                    n0 = min(15, xx) - max(0, xx - 2) + 1
                    coffs[xx + 1] = coffs[xx] + 4 * n0 * O
                xx_order = [x for s in range(xxs) for x in range(s, 18, xxs)]
                for xx in xx_order:
                    coff = coffs[xx]
                    x_lo, x_hi = max(0, xx - 2), min(15, xx)
                    n = x_hi - x_lo + 1
                    for j in range(4):
                        lo = (xwv * XW + 16 * j + xx) * B
                        nc.tensor.matmul(
                            ps[32 * j:32 * j + B,
                               32 * x_lo:32 * (x_hi + 1)],
                            x_t[:, lo:lo + B],
                            w_t[:, r * WCH + coff + j * n * O:
                                 r * WCH + coff + (j + 1) * n * O],
                            start=False,
                            stop=True,
                            skip_group_check=True,
                            tile_position=(0, 32 * j),
                        )
                t1 = tp.tile([128, 512], mybir.dt.float32, tag="t1")
                if elu2:
                    # ELU = max(z, min(exp(z),1) - 1): ACT reads PSUM first,
                    # shortening the exposed per-wave chain
                    nc.scalar.activation(
                        t1[:], ps[:], mybir.ActivationFunctionType.Exp
                    )
                    nc.vector.tensor_scalar(
                        t1[:], t1[:], 1.0, -1.0,
                        op0=mybir.AluOpType.min,
                        op1=mybir.AluOpType.add,
                    )
                    nc.vector.tensor_tensor(
                        out_t[:, wv * 512:(wv + 1) * 512],
                        t1[:], ps[:], op=mybir.AluOpType.max,
                    )
                else:
                    # ELU: out = max(psum, exp(min(psum, 0)) - 1)
                    nc.vector.tensor_scalar_min(t1[:], ps[:], 0.0)
                    nc.scalar.activation(
                        t1[:], t1[:], mybir.ActivationFunctionType.Exp
                    )
                    nc.vector.scalar_tensor_tensor(
                        out_t[:, wv * 512:(wv + 1) * 512],
                        t1[:],
                        -1.0,
                        ps[:],
                        op0=mybir.AluOpType.add,
                        op1=mybir.AluOpType.max,
                    )
                if sowv and wv == sowv:
                    oap = out_d.ap()
                    half = (sowv + 1) * 512
                    for j in range(4):
                        seng.dma_start(
                            oap[j][:, :half], out_t[32 * j:32 * j + 16, :half]
                        )
            oap = out_d.ap()
            half = (sowv + 1) * 512 if sowv else 0
            for j in range(4):
                # split the tail stores across both HWDGE queues
                eng = (nc.sync, nc.scalar)[j % 2]
                eng.dma_start(
                    oap[j][:, half:], out_t[32 * j:32 * j + 16, half:]
                )

    nc.compile()
    if hw:
        nc.m = get_hw_module(nc.m)
    return nc


def _pack_inputs(x, weights, wdt="f8e3"):
    """Host-side scatter: per-core slabs (bf16 patch, fp8/bf16 weights)."""
    np_wdt = {"f8e3": F8E3, "bf16": BF16}[wdt]
    wsc = WSCALE if wdt == "f8e3" else 1.0
    xsc = np.float32(1.0 / wsc)
    xpad = np.pad(np.asarray(x, np.float32) * xsc,
                  ((0, 0), (0, 0), (1, 1), (1, 1))).astype(BF16)  # [B,C,66,66]
    wb = (np.asarray(weights, np.float32) * wsc).astype(np_wdt)  # [O,C,3,3,H,W]
    in_maps = []
    for k in range(NCORES):
        # x slab: [di*32+c, y, xx, b] = xpad[b, c, 8k+y+di, xx]
        slabs = [
            np.transpose(xpad[:, :, 8 * k + di:8 * k + di + HL, :], (1, 2, 3, 0))
            for di in range(KW)
        ]
        xs_k = np.ascontiguousarray(np.stack(slabs, 0)).reshape(PART, XFREE)
        # weights, merged-xx layout: per (y, xx, j), 32-col blocks for
        # x = x_lo..x_hi ascending (dj = xx-x descending):
        #   block = W[o, c, di, dj, 8k+y, 16j+x] as [di*32+c, y, o]
        wc = np.transpose(
            wb[:, :, :, :, 8 * k:8 * (k + 1), :], (2, 1, 3, 4, 5, 0)
        )  # [di, c, dj, y, x, o]
        wc = wc.reshape(PART, KW, HL, W, O)  # [(di,c), dj, y, x, o]
        w_k = np.empty((PART, HL, WCH), dtype=np_wdt)
        coff = 0
        for xx in range(18):
            x_lo, x_hi = max(0, xx - 2), min(15, xx)
            n = x_hi - x_lo + 1
            for j in range(4):
                for t, xr in enumerate(range(x_lo, x_hi + 1)):
                    dj = xx - xr
                    c0 = coff + j * n * O + t * O
                    # [(di,c), y, o]
                    w_k[:, :, c0:c0 + O] = wc[:, dj, :, 16 * j + xr, :]
            coff += 4 * n * O
        in_maps.append({"xs": xs_k, "w": w_k.reshape(PART, WFREE)})
    return in_maps


def _unpack_outputs(results):
    out = np.empty((B, O, H, W), dtype=np.float32)
    for k in range(NCORES):
        arr = results[k]["out"].astype(np.float32)
        arr = arr.reshape(4, 16, HL, 16, O)  # [j, b, w, slot, o]
        strip = np.transpose(arr, (1, 4, 2, 0, 3)).reshape(B, O, HL, W)
        out[:, :, 8 * k:8 * (k + 1), :] = strip
    return out


def run(x, weights, trace=False, **bkw):
    key = tuple(sorted(bkw.items()))
    if key not in _CACHE:
        _CACHE[key] = _build(**bkw)
    nc = _CACHE[key]
    in_maps = _pack_inputs(np.asarray(x), np.asarray(weights),
                           wdt=bkw.get("wdt", "f8e3"))
    res = run_bass_kernel_spmd(nc, in_maps, list(range(NCORES)), trace=trace)
    return _unpack_outputs(res.results), res


def kernel(x, weights):
    out, _ = run(x, weights)
    return out


# revision 30
# speedup vs baseline: 264475.0806x; 1.0049x over previous
"""Locally-connected 3x3 block (LCBlock) Trainium2 kernel.

Computes out = ELU(einsum('ocdkij,bcdkij->boij', weights, unfold(x)))
for x:[16,32,64,64] f32, weights:[32,32,3,3,64,64] f32.

Strategy (8 NeuronCores, SPMD, no collectives):
  - Spatially shard H=64 into 8 strips of 8 rows; each core gets its strip's
    per-position weights (they shard perfectly) and a 10-row halo'd slab of x.
  - Per position p=(y,x) the LC contraction is a tiny matmul
    [B=16, CK=288] x [CK=288, O=32].  We run it on the PE as 3 PSUM-accumulated
    matmuls (one per dj kernel column): lhsT = patch [K=96=(3di x 32c), M=16b]
    (cheap LDWEIGHTS: cost scales with columns=16), rhs = weights
    [96, 32o] (the big tensor streams as the moving operand).  4 positions run
    concurrently in the 4 PE column-groups via tile_position.
  - Weights stream as fp8 e3m4 (x128 host-side scale; patch bf16 carries the
    1/128 descale) -> halves the dominant HBM stream vs bf16.  Output in bf16.
  - All input DMAs issue up front in per-wave chunks (whole fp8 slab fits
    SBUF), x in quarter tiles interleaved, so each wave's gate opens as early
    as possible and the stream never waits on compute.  Each DMA completion
    sem fires ~2us after its last byte (write-receipt straggler), so chunk
    pacing is matched to the ~2.8us/wave Tensor pipeline.
  - One PSUM bank per wave, all zeroed up front (a per-wave memset would
    queue behind the previous wave's ELU on DVE and serialize the pipeline).
  - ELU = max(z, min(exp(z),1)-1): ACT reads PSUM directly, then 2 DVE ops.
  - Stores overlap compute; dispatched from the Sync engine so they never
    delay ACT's exp in the ELU chain.
Host side packs/scatters inputs and gathers the 8 output strips.
"""

import os
import sys

import numpy as np

for _p in ("/opt/trn_rl_repo", "/root/.axon_site/_ro/trn_rl_repo"):
    if os.path.isdir(_p) and _p not in sys.path:
        sys.path.insert(0, _p)

import ml_dtypes

import concourse.bacc as bacc
import concourse.mybir as mybir
import concourse.tile as tile
from concourse.bass_interp import get_hw_module
from concourse.bass_utils import run_bass_kernel_spmd

BF16 = ml_dtypes.bfloat16
F8E3 = ml_dtypes.float8_e3m4
WSCALE = 128.0  # weights *128 into e3m4 normal range; patch /128 compensates

# Problem shape (hardcoded per contract).
B, C, O, H, W = 16, 32, 32, 64, 64
NCORES = 8
HL = H // NCORES  # local rows per core
KW = 3  # conv kernel size
PART = KW * C  # 96 partitions: (di, c)
XW = W + 2  # padded row width
XFREE = HL * XW * B  # x slab free elems/partition
WCH = 4 * 16 * KW * O  # weight elems/partition per row-wave (j, pbl, dj, o)
WFREE = HL * WCH
OUTF = HL * 16 * O  # out free elems/partition: (w, pbl, o)

_CACHE = {}


def _build(hw=True, chunks=(1, 1, 1, 1, 1, 1, 1, 1), wdt="f8e3", odt="bf16",
           upfront=True, sowv=5, store_eng="sync", ms_eng="vector", reps=1,
           xxs=1, elu2=True, xsplit=4):
    w_dt = {"f8e3": mybir.dt.float8e3, "bf16": mybir.dt.bfloat16}[wdt]
    o_dt = {"bf16": mybir.dt.bfloat16, "f32": mybir.dt.float32}[odt]
    nc = bacc.Bacc(
        "TRN2", target_bir_lowering=False, debug=False, num_devices=NCORES
    )
    xs_d = nc.dram_tensor("xs", [PART, XFREE], mybir.dt.bfloat16, kind="ExternalInput")
    w_d = nc.dram_tensor("w", [PART, WFREE], w_dt, kind="ExternalInput")
    out_d = nc.dram_tensor("out", [4, 16, OUTF], o_dt, kind="ExternalOutput")

    assert sum(chunks) == HL
    nchunk = len(chunks)
    cstart = [sum(chunks[:g]) for g in range(nchunk + 1)]
    with tile.TileContext(nc) as tc:
        with (
            tc.tile_pool(name="xp", bufs=1) as xp,
            tc.tile_pool(name="wp", bufs=(1 if upfront else 3)) as wp,
            tc.tile_pool(name="pp", bufs=1, space="PSUM") as pp,
            tc.tile_pool(name="op", bufs=1) as op,
            tc.tile_pool(name="tp", bufs=3) as tp,
        ):
          for _rep in range(reps):
            seng = {"scalar": nc.scalar, "sync": nc.sync}[store_eng]
            mseng = {"vector": nc.vector, "gpsimd": nc.gpsimd,
                     "scalar": nc.scalar}[ms_eng]
            # x split into xsplit tiles, each DMA'd just before the first w
            # chunk whose waves need it: waves start as early as possible
            xwaves = HL // xsplit  # waves per x tile
            xh = XFREE // xsplit
            x_ts = [xp.tile([PART, xh], mybir.dt.bfloat16, tag=f"x{h}",
                            name=f"x{h}")
                    for h in range(xsplit)]
            xdone = 0

            def _need_x(upto):
                nonlocal xdone
                while xdone * xwaves < upto:
                    nc.sync.dma_start(
                        x_ts[xdone][:],
                        xs_d[:][:, xdone * xh:(xdone + 1) * xh],
                    )
                    xdone += 1

            out_t = op.tile([128, OUTF], o_dt, tag="o")

            w_ts = []
            for wg in range(nchunk):
                _need_x(cstart[wg] + 1)
                w_t = wp.tile([PART, chunks[wg] * WCH], w_dt,
                              tag=f"w{wg}", name=f"w{wg}")
                wlo = cstart[wg] * WCH
                nc.sync.dma_start(
                    w_t[:], w_d[:][:, wlo:wlo + chunks[wg] * WCH]
                )
                w_ts.append(w_t)
            _need_x(HL)

            # all PSUM banks zeroed up front: per-wave matmuls never wait on
            # a memset queued behind the previous wave's ELU ops
            ps_ts = []
            for wv in range(HL):
                ps = pp.tile([128, 512], mybir.dt.float32, tag=f"ps{wv}",
                             name=f"ps{wv}")
                mseng.memset(ps[:], 0.0)
                ps_ts.append(ps)

            for wg in range(nchunk):
              w_t = w_ts[wg]
              for r in range(chunks[wg]):
                wv = cstart[wg] + r
                ps = ps_ts[wv]
                x_t = x_ts[wv // xwaves]
                xwv = wv % xwaves
                # one MM per (xx, j): patch col xx serves dj=0,1,2 for
                # positions x = xx, xx-1, xx-2 (adjacent PSUM slots).
                # stride-xxs xx order keeps consecutive same-quadrant MMs on
                # disjoint PSUM columns (no overlapping-accumulate drains)
                coffs = [0] * 19
                for xx in range(18):
                    n0 = min(15, xx) - max(0, xx - 2) + 1
                    coffs[xx + 1] = coffs[xx] + 4 * n0 * O
                xx_order = [x for s in range(xxs) for x in range(s, 18, xxs)]
                for xx in xx_order:
                    coff = coffs[xx]
                    x_lo, x_hi = max(0, xx - 2), min(15, xx)
                    n = x_hi - x_lo + 1
                    for j in range(4):
                        lo = (xwv * XW + 16 * j + xx) * B
                        nc.tensor.matmul(
                            ps[32 * j:32 * j + B,
                               32 * x_lo:32 * (x_hi + 1)],
                            x_t[:, lo:lo + B],
                            w_t[:, r * WCH + coff + j * n * O:
                                 r * WCH + coff + (j + 1) * n * O],
                            start=False,
                            stop=True,
                            skip_group_check=True,
                            tile_position=(0, 32 * j),
                        )
                t1 = tp.tile([128, 512], mybir.dt.float32, tag="t1")
                if elu2:
                    # ELU = max(z, min(exp(z),1) - 1): ACT reads PSUM first,
                    # shortening the exposed per-wave chain
                    nc.scalar.activation(
                        t1[:], ps[:], mybir.ActivationFunctionType.Exp
                    )
                    nc.vector.tensor_scalar(
                        t1[:], t1[:], 1.0, -1.0,
                        op0=mybir.AluOpType.min,
                        op1=mybir.AluOpType.add,
                    )
                    nc.vector.tensor_tensor(
                        out_t[:, wv * 512:(wv + 1) * 512],
                        t1[:], ps[:], op=mybir.AluOpType.max,
                    )
                else:
                    # ELU: out = max(psum, exp(min(psum, 0)) - 1)
                    nc.vector.tensor_scalar_min(t1[:], ps[:], 0.0)
                    nc.scalar.activation(
                        t1[:], t1[:], mybir.ActivationFunctionType.Exp
                    )
                    nc.vector.scalar_tensor_tensor(
                        out_t[:, wv * 512:(wv + 1) * 512],
                        t1[:],
                        -1.0,
                        ps[:],
                        op0=mybir.AluOpType.add,
                        op1=mybir.AluOpType.max,
                    )
                if sowv and wv == sowv:
                    oap = out_d.ap()
                    half = (sowv + 1) * 512
                    for j in range(4):
                        seng.dma_start(
                            oap[j][:, :half], out_t[32 * j:32 * j + 16, :half]
                        )
            oap = out_d.ap()
            half = (sowv + 1) * 512 if sowv else 0
            for j in range(4):
                # split the tail stores across both HWDGE queues
                eng = (nc.sync, nc.scalar)[j % 2]
                eng.dma_start(
                    oap[j][:, half:], out_t[32 * j:32 * j + 16, half:]
                )

    nc.compile()
    if hw:
        nc.m = get_hw_module(nc.m)
    return nc


def _pack_inputs(x, weights, wdt="f8e3"):
    """Host-side scatter: per-core slabs (bf16 patch, fp8/bf16 weights)."""
    np_wdt = {"f8e3": F8E3, "bf16": BF16}[wdt]
    wsc = WSCALE if wdt == "f8e3" else 1.0
    xsc = np.float32(1.0 / wsc)
    xpad = np.pad(np.asarray(x, np.float32) * xsc,
                  ((0, 0), (0, 0), (1, 1), (1, 1))).astype(BF16)  # [B,C,66,66]
    wb = (np.asarray(weights, np.float32) * wsc).astype(np_wdt)  # [O,C,3,3,H,W]
    in_maps = []
    for k in range(NCORES):
        # x slab: [di*32+c, y, xx, b] = xpad[b, c, 8k+y+di, xx]
        slabs = [
            np.transpose(xpad[:, :, 8 * k + di:8 * k + di + HL, :], (1, 2, 3, 0))
            for di in range(KW)
        ]
        xs_k = np.ascontiguousarray(np.stack(slabs, 0)).reshape(PART, XFREE)
        # weights, merged-xx layout: per (y, xx, j), 32-col blocks for
        # x = x_lo..x_hi ascending (dj = xx-x descending):
        #   block = W[o, c, di, dj, 8k+y, 16j+x] as [di*32+c, y, o]
        wc = np.transpose(
            wb[:, :, :, :, 8 * k:8 * (k + 1), :], (2, 1, 3, 4, 5, 0)
        )  # [di, c, dj, y, x, o]
        wc = wc.reshape(PART, KW, HL, W, O)  # [(di,c), dj, y, x, o]
        w_k = np.empty((PART, HL, WCH), dtype=np_wdt)
        coff = 0
        for xx in range(18):
            x_lo, x_hi = max(0, xx - 2), min(15, xx)
            n = x_hi - x_lo + 1
            for j in range(4):
                for t, xr in enumerate(range(x_lo, x_hi + 1)):
                    dj = xx - xr
                    c0 = coff + j * n * O + t * O
                    # [(di,c), y, o]
                    w_k[:, :, c0:c0 + O] = wc[:, dj, :, 16 * j + xr, :]
            coff += 4 * n * O
        in_maps.append({"xs": xs_k, "w": w_k.reshape(PART, WFREE)})
    return in_maps


def _unpack_outputs(results):
    out = np.empty((B, O, H, W), dtype=np.float32)
    for k in range(NCORES):
        arr = results[k]["out"].astype(np.float32)
        arr = arr.reshape(4, 16, HL, 16, O)  # [j, b, w, slot, o]
        strip = np.transpose(arr, (1, 4, 2, 0, 3)).reshape(B, O, HL, W)
        out[:, :, 8 * k:8 * (k + 1), :] = strip
    return out


def run(x, weights, trace=False, **bkw):
    key = tuple(sorted(bkw.items()))
    if key not in _CACHE:
        _CACHE[key] = _build(**bkw)
    nc = _CACHE[key]
    in_maps = _pack_inputs(np.asarray(x), np.asarray(weights),
                           wdt=bkw.get("wdt", "f8e3"))
    res = run_bass_kernel_spmd(nc, in_maps, list(range(NCORES)), trace=trace)
    return _unpack_outputs(res.results), res


def kernel(x, weights):
    out, _ = run(x, weights)
    return out
